# revision 13
# baseline (speedup 1.0000x reference)
"""Trainium2 Bass kernel for nn_Denoiser (6-layer dense transformer denoiser).

Strategy (8 NeuronCores):
  - Token-parallel: the 2048 tokens (B=2 x T=1024) are split 256/core.
    Cores 0-3 hold batch 0, cores 4-7 hold batch 1.
  - All per-token compute (matmuls, layernorms, MLP) is local to a core.
    Attention needs all keys/values of the core's batch, so each layer does
    one grouped AllGather of K plus one of V^T among the 4 cores of a batch.
  - Weights are replicated per core, pre-cast to bf16 on the host and
    streamed from HBM.  Activations stay SBUF-resident in [feature, token]
    layout so matmuls chain without transposes (out = W.T @ x_T).
  - LayerNorm gains/biases are folded into the following weight on the host;
    on-device LN is (x - mu) * rsig with stats from ones-vector matmuls
    (partition-dim reductions on the PE).
  - Softmax skips max-subtraction (scores are in [-3, 3] for this model) and
    gets its denominator for free from a ones-column appended to V^T.
"""
import sys

sys.path.insert(0, "/opt/trn_rl_repo")

import math

import numpy as np
import ml_dtypes

import concourse.bass as bass
import concourse.bacc as bacc
import concourse.tile as tile
from concourse import mybir
from concourse import bass_utils
from concourse.masks import make_identity

F32 = mybir.dt.float32
BF16 = mybir.dt.bfloat16
AF = mybir.ActivationFunctionType
ALU = mybir.AluOpType

B, T, L = 2, 1024, 6
E, D, H, M = 512, 768, 12, 4
DH = D // H          # 64
N = 256              # tokens per core
P = 128
NC = 8               # cores
G = 4                # cores per batch group
KT_E, KT_D, KT_MD = E // P, D // P, (M * D) // P   # 4, 6, 24
MT_D, MT_3D, MT_MD, MT_E = D // P, 3 * D // P, (M * D) // P, E // P  # 6, 18, 24, 4
NKEY = T // P        # 8 key tiles of 128
REPLICA_GROUPS = [[0, 1, 2, 3], [4, 5, 6, 7]]

_CACHE = {}
DEBUG_TAPS = False


def _bf(x):
    return np.ascontiguousarray(x).astype(ml_dtypes.bfloat16)


def _f32(x):
    return np.ascontiguousarray(x).astype(np.float32)


# ---------------------------------------------------------------------------
# device program
# ---------------------------------------------------------------------------

def build_program():
    nc = bacc.Bacc(
        "TRN2", target_bir_lowering=False, debug=False,
        enable_asserts=True, num_devices=NC,
    )

    xT = nc.dram_tensor("xT", [E, N], F32, kind="ExternalInput").ap()
    w_up = nc.dram_tensor("w_up", [L, E, D], BF16, kind="ExternalInput").ap()
    b_up = nc.dram_tensor("b_up", [L, D], F32, kind="ExternalInput").ap()
    w_at = nc.dram_tensor("w_at", [L, D, 3 * D], BF16, kind="ExternalInput").ap()
    b_at = nc.dram_tensor("b_at", [L, 3 * D], F32, kind="ExternalInput").ap()
    w_ap = nc.dram_tensor("w_ap", [L, D, D], BF16, kind="ExternalInput").ap()
    b_ap = nc.dram_tensor("b_ap", [L, D], F32, kind="ExternalInput").ap()
    w_fc = nc.dram_tensor("w_fc", [L, D, M * D], BF16, kind="ExternalInput").ap()
    b_fc = nc.dram_tensor("b_fc", [L, M * D], F32, kind="ExternalInput").ap()
    w_fp = nc.dram_tensor("w_fp", [L, M * D, D], BF16, kind="ExternalInput").ap()
    b_fp = nc.dram_tensor("b_fp", [L, D], F32, kind="ExternalInput").ap()
    w_dn = nc.dram_tensor("w_dn", [L, D, E], BF16, kind="ExternalInput").ap()
    b_dn = nc.dram_tensor("b_dn", [L, E], F32, kind="ExternalInput").ap()
    w_sm = nc.dram_tensor("w_sm", [E, E], BF16, kind="ExternalInput").ap()
    b_sm = nc.dram_tensor("b_sm", [E], F32, kind="ExternalInput").ap()
    outT = nc.dram_tensor("outT", [E, N], F32, kind="ExternalOutput").ap()

    with tile.TileContext(nc) as tc:
        _body(tc, dict(
            xT=xT, w_up=w_up, b_up=b_up, w_at=w_at, b_at=b_at, w_ap=w_ap,
            b_ap=b_ap, w_fc=w_fc, b_fc=b_fc, w_fp=w_fp, b_fp=b_fp,
            w_dn=w_dn, b_dn=b_dn, w_sm=w_sm, b_sm=b_sm, outT=outT,
        ))
    nc.compile()
    return nc


def _body(tc, io):
    nc = tc.nc

    con = tc.alloc_tile_pool(name="con", bufs=1)           # constants / biases
    wgt = tc.alloc_tile_pool(name="wgt", bufs=1)           # streamed weights
    act = tc.alloc_tile_pool(name="act", bufs=1)           # activations
    st = tc.alloc_tile_pool(name="st", bufs=1)             # [1,256] stats
    dram = tc.alloc_tile_pool(name="dram", bufs=1, space="DRAM")
    pbig = tc.alloc_tile_pool(name="pbig", bufs=1, space="PSUM")
    psml = tc.alloc_tile_pool(name="psml", bufs=1, space="PSUM")

    # --- constants ----------------------------------------------------------
    ident = con.tile([P, P], BF16, tag="ident", name="ident")
    make_identity(nc, ident)
    ones_col = con.tile([P, 1], BF16, tag="ones_col", name="ones_col")
    nc.vector.memset(ones_col, 1.0)
    ones_row = con.tile([1, P], F32, tag="ones_row", name="ones_row")
    nc.vector.memset(ones_row, 1.0)
    ones_row_bf = con.tile([1, DH], BF16, tag="ones_row_bf", name="ones_row_bf")
    nc.vector.memset(ones_row_bf, 1.0)
    eps = con.tile([1, 1], F32, tag="eps", name="eps")
    nc.vector.memset(eps, 1e-5)

    # --- biases (resident, f32) --------------------------------------------
    def load_bias(name, dram_ap, nm):
        t = con.tile([P, L * nm], F32, tag=name, name=name)
        nc.sync.dma_start(out=t, in_=dram_ap.rearrange("l (m p) -> p (l m)", p=P))
        return t

    bup = load_bias("bup", io["b_up"], MT_D)
    bat = load_bias("bat", io["b_at"], MT_3D)
    bap = load_bias("bap", io["b_ap"], MT_D)
    bfc = load_bias("bfc", io["b_fc"], MT_MD)
    bfp = load_bias("bfp", io["b_fp"], MT_D)
    bdn = load_bias("bdn", io["b_dn"], MT_E)
    bsm = con.tile([P, MT_E], F32, tag="bsm", name="bsm")
    nc.sync.dma_start(out=bsm, in_=io["b_sm"].rearrange("(m p) -> p m", p=P))

    # --- weight streaming ---------------------------------------------------
    def load_w(dram3, l, kt, c0, c1, tag, bufs):
        t = wgt.tile([P, c1 - c0], BF16, tag=tag, bufs=bufs,
                     name=f"{tag}_{l}_{kt}_{c0}")
        d2 = dram3[l] if len(dram3.shape) == 3 else dram3
        src = d2.rearrange("(kt p) n -> kt p n", p=P)[kt, :, c0:c1]
        nc.sync.dma_start(out=t, in_=src)
        return t

    # --- LN helper ----------------------------------------------------------
    def layer_norm(src, nt, z, tag):
        """z (bf16 [P, nt, N]) = (src - mu) * rsig, stats over nt*P features."""
        ps_sum = psml.tile([1, N], F32, tag="psml", bufs=2, name=f"ps_sum_{tag}")
        ps_sq = psml.tile([1, N], F32, tag="psml", bufs=2, name=f"ps_sq_{tag}")
        for kt in range(nt):
            hb = act.tile([P, N], BF16, tag="lnb", bufs=2, name=f"lnb_{tag}_{kt}")
            nc.vector.tensor_copy(out=hb, in_=src[:, kt, :])
            nc.tensor.matmul(ps_sum, ones_col, hb, start=(kt == 0), stop=(kt == nt - 1))
            sq = act.tile([P, N], BF16, tag="lnsq", bufs=2, name=f"lnsq_{tag}_{kt}")
            nc.scalar.activation(out=sq, in_=hb, func=AF.Square)
            nc.tensor.matmul(ps_sq, ones_col, sq, start=(kt == 0), stop=(kt == nt - 1))
        dinv = 1.0 / (nt * P)
        mu = st.tile([1, N], F32, tag="st", bufs=5, name=f"mu_{tag}")
        nc.vector.tensor_scalar_mul(out=mu, in0=ps_sum, scalar1=dinv)
        ms = st.tile([1, N], F32, tag="st", bufs=5, name=f"ms_{tag}")
        nc.vector.tensor_scalar_mul(out=ms, in0=ps_sq, scalar1=dinv)
        # var = ms - mu^2
        var = st.tile([1, N], F32, tag="st", bufs=5, name=f"var_{tag}")
        mu2 = st.tile([1, N], F32, tag="st", bufs=5, name=f"mu2_{tag}")
        nc.vector.tensor_mul(out=mu2, in0=mu, in1=mu)
        nc.vector.tensor_sub(out=var, in0=ms, in1=mu2)
        sd = st.tile([1, N], F32, tag="st", bufs=5, name=f"sd_{tag}")
        nc.scalar.activation(out=sd, in_=var, func=AF.Sqrt, bias=eps)
        rsig = st.tile([1, N], F32, tag="st", bufs=5, name=f"rsig_{tag}")
        nc.vector.reciprocal(out=rsig, in_=sd)
        ps_rs = pbig.tile([P, N], F32, tag="ps_bc", bufs=2, name=f"ps_rs_{tag}")
        nc.tensor.matmul(ps_rs, ones_row, rsig, start=True, stop=True)
        ps_nm = pbig.tile([P, N], F32, tag="ps_bc", bufs=2, name=f"ps_nm_{tag}")
        nc.tensor.matmul(ps_nm, ones_row, mu, start=True, stop=True)
        for kt in range(nt):
            tmp = act.tile([P, N], F32, tag="lntmp", bufs=2, name=f"lnt_{tag}_{kt}")
            nc.vector.tensor_sub(out=tmp, in0=src[:, kt, :], in1=ps_nm)
            nc.vector.tensor_mul(out=z[:, kt, :], in0=tmp, in1=ps_rs)

    # --- generic matmul phase ----------------------------------------------
    def mm_phase(n_m, n_k, lhsT_fn, rhs_fn, evac_fn, psname):
        for mi in range(n_m):
            ps = pbig.tile([P, N], F32, tag="ps_mm", bufs=4,
                           name=f"ps_{psname}_{mi}")
            for ki in range(n_k):
                nc.tensor.matmul(ps, lhsT_fn(ki, mi), rhs_fn(ki),
                                 start=(ki == 0), stop=(ki == n_k - 1))
            evac_fn(mi, ps)


    def tap(name, t, dtype):
        if not DEBUG_TAPS:
            return
        rows = t.shape[1] * P
        d = nc.dram_tensor(f"dbg_{name}", [rows, N], dtype,
                           kind="ExternalOutput").ap()
        nc.sync.dma_start(out=d.rearrange("(kt p) n -> p kt n", p=P), in_=t)

    # --- input --------------------------------------------------------------
    x = act.tile([P, KT_E, N], F32, tag="x", bufs=2, name="x0")
    nc.sync.dma_start(out=x, in_=io["xT"].rearrange("(kt p) n -> p kt n", p=P))

    for l in range(L):
        # ---- load weights for this layer (emission order ~ compute order)
        up_t = [load_w(io["w_up"], l, kt, 0, D, "wB", 47) for kt in range(KT_E)]
        atk_t = [load_w(io["w_at"], l, kt, D, 2 * D, "wB", 47) for kt in range(KT_D)]
        atv_t = [load_w(io["w_at"], l, kt, 2 * D, 3 * D, "wB", 47) for kt in range(KT_D)]
        atq_t = [load_w(io["w_at"], l, kt, 0, D, "wB", 47) for kt in range(KT_D)]
        app_t = [load_w(io["w_ap"], l, kt, 0, D, "wB", 47) for kt in range(KT_D)]
        fc_t = [[load_w(io["w_fc"], l, kt, c * D, (c + 1) * D, "wB", 47)
                 for kt in range(KT_D)] for c in range(4)]
        fp_t = [load_w(io["w_fp"], l, kt, 0, D, "wB", 47) for kt in range(KT_MD)]
        dn_t = [load_w(io["w_dn"], l, kt, 0, E, "wB", 47) for kt in range(KT_D)]

        # ---- x cast to bf16
        xb = act.tile([P, KT_E, N], BF16, tag="xb", bufs=2, name=f"xb_{l}")
        nc.vector.tensor_copy(out=xb, in_=x)

        # ---- up projection: h1 = up(x) + b
        h1 = act.tile([P, KT_D, N], F32, tag="h", bufs=2, name=f"h1_{l}")

        def up_evac(mi, ps, l=l, h1=h1):
            nc.vector.tensor_scalar_add(
                out=h1[:, mi, :], in0=ps, scalar1=bup[:, l * MT_D + mi: l * MT_D + mi + 1])

        mm_phase(MT_D, KT_E,
                 lambda ki, mi, up_t=up_t: up_t[ki][:, mi * P:(mi + 1) * P],
                 lambda ki, xb=xb: xb[:, ki, :],
                 up_evac, f"up{l}")

        tap(f"h1_{l}", h1, F32)

        # ---- ln1
        z1 = act.tile([P, KT_D, N], BF16, tag="z", bufs=2, name=f"z1_{l}")
        layer_norm(h1, KT_D, z1, f"ln1_{l}")
        tap(f"z1_{l}", z1, BF16)

        # ---- qkv (k first, then v, then q) + AllGathers
        qk = act.tile([P, MT_D, N], BF16, tag="qk", bufs=1, name=f"qk_{l}")
        qv = act.tile([P, MT_D, N], BF16, tag="qv", bufs=1, name=f"qv_{l}")
        qq = act.tile([P, MT_D, N], BF16, tag="qq", bufs=1, name=f"qq_{l}")

        def qkv_evac(base, dst, l=l):
            def f(mi, ps):
                m = base + mi
                nc.vector.tensor_scalar_add(
                    out=dst[:, mi, :], in0=ps,
                    scalar1=bat[:, l * MT_3D + m: l * MT_3D + m + 1])
            return f

        z1rhs = lambda ki, z1=z1: z1[:, ki, :]
        # k part (tiles 6..11)
        mm_phase(MT_D, KT_D,
                 lambda ki, mi, w=atk_t: w[ki][:, mi * P:(mi + 1) * P],
                 z1rhs, qkv_evac(6, qk), f"qk{l}")
        KB = KT_D * P * N
        VB = 2 * P * H * (DH + 1)
        ag_in = dram.tile([KB + VB], BF16, tag="ag_in", bufs=2,
                          name=f"ag_in_{l}")
        nc.gpsimd.dma_start(
            out=ag_in[0:KB].rearrange("(kt p n) -> p kt n", kt=KT_D, p=P),
            in_=qk)

        # v part (tiles 12..17), then transpose chunks + ones column
        mm_phase(MT_D, KT_D,
                 lambda ki, mi, w=atv_t: w[ki][:, mi * P:(mi + 1) * P],
                 z1rhs, qkv_evac(12, qv), f"qv{l}")
        vtc = act.tile([P, 2, H, DH + 1], BF16, tag="vtc", bufs=2, name=f"vtc_{l}")
        nc.vector.memset(vtc[:, :, :, DH:DH + 1], 1.0)
        for ft in range(KT_D):
            for tt in range(2):
                ps_t = psml.tile([P, P], BF16, tag="psml", bufs=2,
                                 name=f"ps_t_{l}_{ft}_{tt}")
                nc.tensor.transpose(ps_t, qv[:, ft, tt * P:(tt + 1) * P], ident)
                nc.vector.tensor_copy(out=vtc[:, tt, 2 * ft, 0:DH], in_=ps_t[:, 0:DH])
                nc.vector.tensor_copy(out=vtc[:, tt, 2 * ft + 1, 0:DH], in_=ps_t[:, DH:P])
        nc.gpsimd.dma_start(
            out=ag_in[KB:KB + VB].rearrange("(tt p h c) -> p tt (h c)",
                                            tt=2, p=P, h=H),
            in_=vtc.rearrange("p tt h c -> p tt (h c)"))
        ag_out = dram.tile([G, KB + VB], BF16, tag="ag_out", bufs=2,
                           name=f"ag_out_{l}")
        nc.gpsimd.collective_compute(
            "AllGather", ALU.bypass, replica_groups=REPLICA_GROUPS,
            ins=[ag_in.opt()], outs=[ag_out.opt()])

        # q part (tiles 0..5)
        mm_phase(MT_D, KT_D,
                 lambda ki, mi, w=atq_t: w[ki][:, mi * P:(mi + 1) * P],
                 z1rhs, qkv_evac(0, qq), f"qq{l}")

        # ---- gather K, V^T into SBUF (per-rank / per-keytile granularity)
        kf_r = []
        for r in range(G):
            t = act.tile([P, KT_D, N], BF16, tag="kf", bufs=4, name=f"kf_{l}_{r}")
            nc.gpsimd.dma_start(
                out=t,
                in_=ag_out[r, 0:KB].rearrange("(kt p n) -> p kt n", kt=KT_D, p=P))
            kf_r.append(t)
        vtf_j = []
        for j in range(NKEY):
            t = act.tile([P, H, DH + 1], BF16, tag="vtf", bufs=8,
                         name=f"vtf_{l}_{j}")
            o0 = KB + (j % 2) * P * H * (DH + 1)
            nc.gpsimd.dma_start(
                out=t,
                in_=ag_out[j // 2, o0:o0 + P * H * (DH + 1)].rearrange(
                    "(p h c) -> p h c", p=P, h=H))
            vtf_j.append(t)

        # ---- attention: S^T/exp pipelined ahead of AV by LAG heads
        y = act.tile([P, KT_D, N], BF16, tag="y", bufs=2, name=f"y_{l}")

        def head_st(h, l=l, qq=qq, kf_r=kf_r):
            po = (h % 2) * DH
            ft = h // 2
            q_h = qq[po:po + DH, ft, :]
            expS = act.tile([P, NKEY, N], BF16, tag="expS", bufs=4,
                            name=f"expS_{l}_{h}")
            for j in range(NKEY):
                ps_s = pbig.tile([P, N], F32, tag="ps_mm", bufs=4,
                                 name=f"ps_s_{l}_{h}_{j}")
                k_h = kf_r[j // 2][po:po + DH, ft, (j % 2) * P:(j % 2) * P + P]
                nc.tensor.matmul(ps_s, k_h, q_h, start=True, stop=True)
                nc.scalar.activation(out=expS[:, j, :], in_=ps_s, func=AF.Exp)
            return expS

        def head_av(h, expS, l=l, y=y, vtf_j=vtf_j):
            po = (h % 2) * DH
            ft = h // 2
            ps_y = pbig.tile([DH + 1, N], F32, tag="ps_mm", bufs=4,
                             name=f"ps_y_{l}_{h}")
            for j in range(NKEY):
                nc.tensor.matmul(ps_y, vtf_j[j][:, h, :], expS[:, j, :],
                                 start=(j == 0), stop=(j == NKEY - 1))
            rec = st.tile([1, N], F32, tag="st", bufs=5, name=f"rec_{l}_{h}")
            nc.vector.reciprocal(out=rec, in_=ps_y[DH:DH + 1, :])
            rec_bf = st.tile([1, N], BF16, tag="stbf", bufs=2, name=f"recb_{l}_{h}")
            nc.vector.tensor_copy(out=rec_bf, in_=rec)
            ps_rb = pbig.tile([DH, N], F32, tag="ps_bc", bufs=2,
                              name=f"ps_rb_{l}_{h}")
            nc.tensor.matmul(ps_rb, ones_row_bf, rec_bf, start=True, stop=True)
            rb = act.tile([DH, N], BF16, tag="rb", bufs=2, name=f"rb_{l}_{h}")
            nc.scalar.activation(out=rb, in_=ps_rb, func=AF.Copy)
            nc.vector.tensor_tensor(out=y[po:po + DH, ft, :], in0=ps_y[0:DH, :],
                                    in1=rb, op=ALU.mult)

        LAG = 3
        pend = {}
        for h in range(H):
            pend[h] = head_st(h)
            if h >= LAG:
                head_av(h - LAG, pend.pop(h - LAG))
        for h in range(H - LAG, H):
            head_av(h, pend.pop(h))

        tap(f"y_{l}", y, BF16)

        # ---- attn out projection + residual: h2 = attnp(y) + b + h1
        h2 = act.tile([P, KT_D, N], F32, tag="h", bufs=2, name=f"h2_{l}")

        def ap_evac(mi, ps, l=l, h1=h1, h2=h2):
            nc.vector.scalar_tensor_tensor(
                out=h2[:, mi, :], in0=ps,
                scalar=bap[:, l * MT_D + mi: l * MT_D + mi + 1],
                in1=h1[:, mi, :], op0=ALU.add, op1=ALU.add)

        mm_phase(MT_D, KT_D,
                 lambda ki, mi, w=app_t: w[ki][:, mi * P:(mi + 1) * P],
                 lambda ki, y=y: y[:, ki, :],
                 ap_evac, f"ap{l}")

        tap(f"h2_{l}", h2, F32)

        # ---- ln2
        z2 = act.tile([P, KT_D, N], BF16, tag="z", bufs=2, name=f"z2_{l}")
        layer_norm(h2, KT_D, z2, f"ln2_{l}")

        # ---- MLP fc + gelu
        mg = act.tile([P, MT_MD, N], BF16, tag="mg", bufs=1, name=f"mg_{l}")

        def fc_evac(mi, ps, l=l, mg=mg):
            nc.scalar.activation(
                out=mg[:, mi, :], in_=ps, func=AF.Gelu_apprx_tanh,
                bias=bfc[:, l * MT_MD + mi: l * MT_MD + mi + 1])

        mm_phase(MT_MD, KT_D,
                 lambda ki, mi, fc_t=fc_t: fc_t[mi // MT_D][ki][:, (mi % MT_D) * P:(mi % MT_D + 1) * P],
                 lambda ki, z2=z2: z2[:, ki, :],
                 fc_evac, f"fc{l}")

        # ---- MLP proj + residual (h3 kept bf16; only feeds the down matmul)
        h3 = act.tile([P, KT_D, N], BF16, tag="h3", bufs=2, name=f"h3_{l}")

        def fp_evac(mi, ps, l=l, h2=h2, h3=h3):
            nc.vector.scalar_tensor_tensor(
                out=h3[:, mi, :], in0=ps,
                scalar=bfp[:, l * MT_D + mi: l * MT_D + mi + 1],
                in1=h2[:, mi, :], op0=ALU.add, op1=ALU.add)

        mm_phase(MT_D, KT_MD,
                 lambda ki, mi, w=fp_t: w[ki][:, mi * P:(mi + 1) * P],
                 lambda ki, mg=mg: mg[:, ki, :],
                 fp_evac, f"fp{l}")

        tap(f"mg_{l}", mg, BF16)
        tap(f"h3_{l}", h3, BF16)

        # ---- down projection + residual into x
        x_new = act.tile([P, KT_E, N], F32, tag="x", bufs=2, name=f"x_{l + 1}")

        def dn_evac(mi, ps, l=l, x=x, x_new=x_new):
            nc.vector.scalar_tensor_tensor(
                out=x_new[:, mi, :], in0=ps,
                scalar=bdn[:, l * MT_E + mi: l * MT_E + mi + 1],
                in1=x[:, mi, :], op0=ALU.add, op1=ALU.add)

        mm_phase(MT_E, KT_D,
                 lambda ki, mi, w=dn_t: w[ki][:, mi * P:(mi + 1) * P],
                 lambda ki, h3=h3: h3[:, ki, :],
                 dn_evac, f"dn{l}")
        tap(f"x_{l + 1}", x_new, F32)
        x = x_new

    # ---- final layernorm + small head
    zf = act.tile([P, KT_E, N], BF16, tag="z", bufs=2, name="zf")
    layer_norm(x, KT_E, zf, "lnf")
    sm_t = [load_w(io["w_sm"], 0, kt, 0, E, "wB", 47) for kt in range(KT_E)]
    osb = act.tile([P, MT_E, N], F32, tag="osb", bufs=1, name="osb")

    def sm_evac(mi, ps, osb=osb):
        nc.vector.tensor_scalar_add(out=osb[:, mi, :], in0=ps,
                                    scalar1=bsm[:, mi:mi + 1])

    mm_phase(MT_E, KT_E,
             lambda ki, mi, w=sm_t: w[ki][:, mi * P:(mi + 1) * P],
             lambda ki, zf=zf: zf[:, ki, :],
             sm_evac, "sm")
    nc.sync.dma_start(out=io["outT"].rearrange("(m p) n -> p m n", p=P), in_=osb)

    for pool in (psml, pbig, dram, st, act, wgt, con):
        pool.release()


# ---------------------------------------------------------------------------
# host side
# ---------------------------------------------------------------------------

def _erf(x):
    return np.vectorize(math.erf)(x.astype(np.float64)).astype(np.float32)


def _prepare(inputs):
    """Host preprocessing: fold embeddings + LN gains, cast weights to bf16."""
    ie = _f32(inputs["input_embeddings"])
    wpe = _f32(inputs["wpe"])
    ts = _f32(inputs["time_step"])
    time_w = _f32(inputs["time_w"])
    time_b = _f32(inputs["time_b"])

    half = E // 2
    freqs = np.exp(np.arange(half, dtype=np.float32) * (-math.log(10000.0) / (half - 1)))
    emb = ts[:, None] * freqs[None, :]
    emb = np.concatenate([np.sin(emb), np.cos(emb)], axis=-1).astype(np.float32)
    pre = emb @ time_w + time_b
    te = pre * 0.5 * (1.0 + _erf(pre / np.sqrt(2.0)))
    x0 = ie + wpe[None] + te[:, None, :]           # [B, T, E]

    ln1_g, ln1_b = _f32(inputs["ln1_g"]), _f32(inputs["ln1_b"])
    ln2_g, ln2_b = _f32(inputs["ln2_g"]), _f32(inputs["ln2_b"])
    lnf_g, lnf_b = _f32(inputs["lnf_g"]), _f32(inputs["lnf_b"])
    attn_w, attn_b = _f32(inputs["attn_w"]), _f32(inputs["attn_b"])
    fc_w, fc_b = _f32(inputs["fc_w"]), _f32(inputs["fc_b"])
    small_w, small_b = _f32(inputs["small_w"]), _f32(inputs["small_b"])

    # fold LN gain/bias into the following matmul; scale q by 1/sqrt(DH)
    attn_wf = ln1_g[:, :, None] * attn_w
    attn_bf = attn_b + np.einsum("ld,ldo->lo", ln1_b, attn_w)
    attn_wf[:, :, :D] *= 1.0 / math.sqrt(DH)
    attn_bf[:, :D] *= 1.0 / math.sqrt(DH)
    fc_wf = ln2_g[:, :, None] * fc_w
    fc_bf = fc_b + np.einsum("ld,ldo->lo", ln2_b, fc_w)
    small_wf = lnf_g[:, None] * small_w
    small_bf = small_b + lnf_b @ small_w

    feeds = {
        "w_up": _bf(inputs["up_w"]), "b_up": _f32(inputs["up_b"]),
        "w_at": _bf(attn_wf), "b_at": _f32(attn_bf),
        "w_ap": _bf(inputs["attnp_w"]), "b_ap": _f32(inputs["attnp_b"]),
        "w_fc": _bf(fc_wf), "b_fc": _f32(fc_bf),
        "w_fp": _bf(inputs["fcp_w"]), "b_fp": _f32(inputs["fcp_b"]),
        "w_dn": _bf(inputs["down_w"]), "b_dn": _f32(inputs["down_b"]),
        "w_sm": _bf(small_wf), "b_sm": _f32(small_bf),
    }
    return x0, feeds


def _get_program():
    if "nc" not in _CACHE:
        _CACHE["nc"] = build_program()
    return _CACHE["nc"]


def make_in_maps(inputs):
    x0, feeds = _prepare(inputs)
    in_maps = []
    for c in range(NC):
        b, s = c // G, (c % G) * N
        xT = np.ascontiguousarray(x0[b, s:s + N, :].T)     # [E, N] f32
        in_maps.append({"xT": xT, **feeds})
    return in_maps


def assemble(results):
    out = np.empty((B, T, E), dtype=np.float32)
    for c in range(NC):
        b, s = c // G, (c % G) * N
        out[b, s:s + N, :] = results[c]["outT"].T
    return out


def kernel(**inputs) -> np.ndarray:
    nc = _get_program()
    in_maps = make_in_maps(inputs)
    res = bass_utils.run_bass_kernel_spmd(nc, in_maps, core_ids=list(range(NC)))
    return assemble(res.results)


# revision 14
# speedup vs baseline: 1.0187x; 1.0187x over previous
"""Trainium2 Bass kernel for nn_Denoiser (6-layer dense transformer denoiser).

Strategy (8 NeuronCores):
  - Token-parallel: the 2048 tokens (B=2 x T=1024) are split 256/core.
    Cores 0-3 hold batch 0, cores 4-7 hold batch 1.
  - All per-token compute (matmuls, layernorms, MLP) is local to a core.
    Attention needs all keys/values of the core's batch, so each layer does
    one grouped AllGather of K plus one of V^T among the 4 cores of a batch.
  - Weights are replicated per core, pre-cast to bf16 on the host and
    streamed from HBM.  Activations stay SBUF-resident in [feature, token]
    layout so matmuls chain without transposes (out = W.T @ x_T).
  - LayerNorm gains/biases are folded into the following weight on the host;
    on-device LN is (x - mu) * rsig with stats from ones-vector matmuls
    (partition-dim reductions on the PE).
  - Softmax skips max-subtraction (scores are in [-3, 3] for this model) and
    gets its denominator for free from a ones-column appended to V^T.
"""
import sys

sys.path.insert(0, "/opt/trn_rl_repo")

import math

import numpy as np
import ml_dtypes

import concourse.bass as bass
import concourse.bacc as bacc
import concourse.tile as tile
from concourse import mybir
from concourse import bass_utils
from concourse.masks import make_identity

F32 = mybir.dt.float32
BF16 = mybir.dt.bfloat16
AF = mybir.ActivationFunctionType
ALU = mybir.AluOpType

B, T, L = 2, 1024, 6
E, D, H, M = 512, 768, 12, 4
DH = D // H          # 64
N = 256              # tokens per core
P = 128
NC = 8               # cores
G = 4                # cores per batch group
KT_E, KT_D, KT_MD = E // P, D // P, (M * D) // P   # 4, 6, 24
MT_D, MT_3D, MT_MD, MT_E = D // P, 3 * D // P, (M * D) // P, E // P  # 6, 18, 24, 4
NKEY = T // P        # 8 key tiles of 128
REPLICA_GROUPS = [[0, 1, 2, 3], [4, 5, 6, 7]]

_CACHE = {}
DEBUG_TAPS = False


def _bf(x):
    return np.ascontiguousarray(x).astype(ml_dtypes.bfloat16)


def _f32(x):
    return np.ascontiguousarray(x).astype(np.float32)


# ---------------------------------------------------------------------------
# device program
# ---------------------------------------------------------------------------

def build_program():
    nc = bacc.Bacc(
        "TRN2", target_bir_lowering=False, debug=False,
        enable_asserts=True, num_devices=NC,
    )

    xT = nc.dram_tensor("xT", [E, N], F32, kind="ExternalInput").ap()
    w_up = nc.dram_tensor("w_up", [L, E, D], BF16, kind="ExternalInput").ap()
    b_up = nc.dram_tensor("b_up", [L, D], F32, kind="ExternalInput").ap()
    w_at = nc.dram_tensor("w_at", [L, D, 3 * D], BF16, kind="ExternalInput").ap()
    b_at = nc.dram_tensor("b_at", [L, 3 * D], F32, kind="ExternalInput").ap()
    w_ap = nc.dram_tensor("w_ap", [L, D, D], BF16, kind="ExternalInput").ap()
    b_ap = nc.dram_tensor("b_ap", [L, D], F32, kind="ExternalInput").ap()
    w_fc = nc.dram_tensor("w_fc", [L, D, M * D], BF16, kind="ExternalInput").ap()
    b_fc = nc.dram_tensor("b_fc", [L, M * D], F32, kind="ExternalInput").ap()
    w_fp = nc.dram_tensor("w_fp", [L, M * D, D], BF16, kind="ExternalInput").ap()
    b_fp = nc.dram_tensor("b_fp", [L, D], F32, kind="ExternalInput").ap()
    w_dn = nc.dram_tensor("w_dn", [L, D, E], BF16, kind="ExternalInput").ap()
    b_dn = nc.dram_tensor("b_dn", [L, E], F32, kind="ExternalInput").ap()
    w_sm = nc.dram_tensor("w_sm", [E, E], BF16, kind="ExternalInput").ap()
    b_sm = nc.dram_tensor("b_sm", [E], F32, kind="ExternalInput").ap()
    outT = nc.dram_tensor("outT", [E, N], F32, kind="ExternalOutput").ap()

    with tile.TileContext(nc) as tc:
        _body(tc, dict(
            xT=xT, w_up=w_up, b_up=b_up, w_at=w_at, b_at=b_at, w_ap=w_ap,
            b_ap=b_ap, w_fc=w_fc, b_fc=b_fc, w_fp=w_fp, b_fp=b_fp,
            w_dn=w_dn, b_dn=b_dn, w_sm=w_sm, b_sm=b_sm, outT=outT,
        ))
    nc.compile()
    return nc


def _body(tc, io):
    nc = tc.nc

    con = tc.alloc_tile_pool(name="con", bufs=1)           # constants / biases
    wgt = tc.alloc_tile_pool(name="wgt", bufs=1)           # streamed weights
    act = tc.alloc_tile_pool(name="act", bufs=1)           # activations
    st = tc.alloc_tile_pool(name="st", bufs=1)             # [1,256] stats
    dram = tc.alloc_tile_pool(name="dram", bufs=1, space="DRAM")
    pbig = tc.alloc_tile_pool(name="pbig", bufs=1, space="PSUM")
    psml = tc.alloc_tile_pool(name="psml", bufs=1, space="PSUM")

    # --- constants ----------------------------------------------------------
    ident = con.tile([P, P], BF16, tag="ident", name="ident")
    make_identity(nc, ident)
    ones_col = con.tile([P, 1], BF16, tag="ones_col", name="ones_col")
    nc.vector.memset(ones_col, 1.0)
    ones_row = con.tile([1, P], F32, tag="ones_row", name="ones_row")
    nc.vector.memset(ones_row, 1.0)
    ones_row_bf = con.tile([1, DH], BF16, tag="ones_row_bf", name="ones_row_bf")
    nc.vector.memset(ones_row_bf, 1.0)
    eps = con.tile([1, 1], F32, tag="eps", name="eps")
    nc.vector.memset(eps, 1e-5)

    # --- biases (resident, f32) --------------------------------------------
    def load_bias(name, dram_ap, nm):
        t = con.tile([P, L * nm], F32, tag=name, name=name)
        nc.sync.dma_start(out=t, in_=dram_ap.rearrange("l (m p) -> p (l m)", p=P))
        return t

    bup = load_bias("bup", io["b_up"], MT_D)
    bat = load_bias("bat", io["b_at"], MT_3D)
    bap = load_bias("bap", io["b_ap"], MT_D)
    bfc = load_bias("bfc", io["b_fc"], MT_MD)
    bfp = load_bias("bfp", io["b_fp"], MT_D)
    bdn = load_bias("bdn", io["b_dn"], MT_E)
    bsm = con.tile([P, MT_E], F32, tag="bsm", name="bsm")
    nc.sync.dma_start(out=bsm, in_=io["b_sm"].rearrange("(m p) -> p m", p=P))

    # --- weight streaming ---------------------------------------------------
    def load_w(dram3, l, kt, c0, c1, tag, bufs):
        t = wgt.tile([P, c1 - c0], BF16, tag=tag, bufs=bufs,
                     name=f"{tag}_{l}_{kt}_{c0}")
        d2 = dram3[l] if len(dram3.shape) == 3 else dram3
        src = d2.rearrange("(kt p) n -> kt p n", p=P)[kt, :, c0:c1]
        nc.sync.dma_start(out=t, in_=src)
        return t

    # --- LN helper ----------------------------------------------------------
    def layer_norm(src, nt, z, tag):
        """z (bf16 [P, nt, N]) = (src - mu) * rsig, stats over nt*P features."""
        ps_sum = psml.tile([1, N], F32, tag="psml", bufs=2, name=f"ps_sum_{tag}")
        ps_sq = psml.tile([1, N], F32, tag="psml", bufs=2, name=f"ps_sq_{tag}")
        for kt in range(nt):
            hb = act.tile([P, N], BF16, tag="lnb", bufs=2, name=f"lnb_{tag}_{kt}")
            nc.vector.tensor_copy(out=hb, in_=src[:, kt, :])
            nc.tensor.matmul(ps_sum, ones_col, hb, start=(kt == 0), stop=(kt == nt - 1))
            sq = act.tile([P, N], BF16, tag="lnsq", bufs=2, name=f"lnsq_{tag}_{kt}")
            nc.scalar.activation(out=sq, in_=hb, func=AF.Square)
            nc.tensor.matmul(ps_sq, ones_col, sq, start=(kt == 0), stop=(kt == nt - 1))
        dinv = 1.0 / (nt * P)
        mu = st.tile([1, N], F32, tag="st", bufs=5, name=f"mu_{tag}")
        nc.vector.tensor_scalar_mul(out=mu, in0=ps_sum, scalar1=dinv)
        ms = st.tile([1, N], F32, tag="st", bufs=5, name=f"ms_{tag}")
        nc.vector.tensor_scalar_mul(out=ms, in0=ps_sq, scalar1=dinv)
        # var = ms - mu^2
        var = st.tile([1, N], F32, tag="st", bufs=5, name=f"var_{tag}")
        mu2 = st.tile([1, N], F32, tag="st", bufs=5, name=f"mu2_{tag}")
        nc.vector.tensor_mul(out=mu2, in0=mu, in1=mu)
        nc.vector.tensor_sub(out=var, in0=ms, in1=mu2)
        sd = st.tile([1, N], F32, tag="st", bufs=5, name=f"sd_{tag}")
        nc.scalar.activation(out=sd, in_=var, func=AF.Sqrt, bias=eps)
        rsig = st.tile([1, N], F32, tag="st", bufs=5, name=f"rsig_{tag}")
        nc.vector.reciprocal(out=rsig, in_=sd)
        ps_rs = pbig.tile([P, N], F32, tag="ps_bc", bufs=2, name=f"ps_rs_{tag}")
        nc.tensor.matmul(ps_rs, ones_row, rsig, start=True, stop=True)
        ps_nm = pbig.tile([P, N], F32, tag="ps_bc", bufs=2, name=f"ps_nm_{tag}")
        nc.tensor.matmul(ps_nm, ones_row, mu, start=True, stop=True)
        for kt in range(nt):
            tmp = act.tile([P, N], F32, tag="lntmp", bufs=2, name=f"lnt_{tag}_{kt}")
            nc.vector.tensor_sub(out=tmp, in0=src[:, kt, :], in1=ps_nm)
            nc.vector.tensor_mul(out=z[:, kt, :], in0=tmp, in1=ps_rs)

    # --- generic matmul phase ----------------------------------------------
    def mm_phase(n_m, n_k, lhsT_fn, rhs_fn, evac_fn, psname):
        for mi in range(n_m):
            ps = pbig.tile([P, N], F32, tag="ps_mm", bufs=4,
                           name=f"ps_{psname}_{mi}")
            for ki in range(n_k):
                nc.tensor.matmul(ps, lhsT_fn(ki, mi), rhs_fn(ki),
                                 start=(ki == 0), stop=(ki == n_k - 1))
            evac_fn(mi, ps)


    def tap(name, t, dtype):
        if not DEBUG_TAPS:
            return
        rows = t.shape[1] * P
        d = nc.dram_tensor(f"dbg_{name}", [rows, N], dtype,
                           kind="ExternalOutput").ap()
        nc.sync.dma_start(out=d.rearrange("(kt p) n -> p kt n", p=P), in_=t)

    # --- input --------------------------------------------------------------
    x = act.tile([P, KT_E, N], F32, tag="x", bufs=2, name="x0")
    nc.sync.dma_start(out=x, in_=io["xT"].rearrange("(kt p) n -> p kt n", p=P))

    for l in range(L):
        # ---- load weights for this layer (emission order ~ compute order)
        up_t = [load_w(io["w_up"], l, kt, 0, D, "wB", 47) for kt in range(KT_E)]
        atk_t = [load_w(io["w_at"], l, kt, D, 2 * D, "wB", 47) for kt in range(KT_D)]
        atv_t = [load_w(io["w_at"], l, kt, 2 * D, 3 * D, "wB", 47) for kt in range(KT_D)]
        atq_t = [load_w(io["w_at"], l, kt, 0, D, "wB", 47) for kt in range(KT_D)]
        app_t = [load_w(io["w_ap"], l, kt, 0, D, "wB", 47) for kt in range(KT_D)]
        fc_t = [[load_w(io["w_fc"], l, kt, c * D, (c + 1) * D, "wB", 47)
                 for kt in range(KT_D)] for c in range(4)]
        fp_t = [load_w(io["w_fp"], l, kt, 0, D, "wB", 47) for kt in range(KT_MD)]
        dn_t = [load_w(io["w_dn"], l, kt, 0, E, "wB", 47) for kt in range(KT_D)]

        # ---- x cast to bf16
        xb = act.tile([P, KT_E, N], BF16, tag="xb", bufs=2, name=f"xb_{l}")
        nc.vector.tensor_copy(out=xb, in_=x)

        # ---- up projection: h1 = up(x) + b
        h1 = act.tile([P, KT_D, N], F32, tag="h", bufs=2, name=f"h1_{l}")

        def up_evac(mi, ps, l=l, h1=h1):
            nc.vector.tensor_scalar_add(
                out=h1[:, mi, :], in0=ps, scalar1=bup[:, l * MT_D + mi: l * MT_D + mi + 1])

        mm_phase(MT_D, KT_E,
                 lambda ki, mi, up_t=up_t: up_t[ki][:, mi * P:(mi + 1) * P],
                 lambda ki, xb=xb: xb[:, ki, :],
                 up_evac, f"up{l}")

        tap(f"h1_{l}", h1, F32)

        # ---- ln1
        z1 = act.tile([P, KT_D, N], BF16, tag="z", bufs=2, name=f"z1_{l}")
        layer_norm(h1, KT_D, z1, f"ln1_{l}")
        tap(f"z1_{l}", z1, BF16)

        # ---- qkv (k first, then v, then q) + AllGathers
        qk = act.tile([P, MT_D, N], BF16, tag="qk", bufs=1, name=f"qk_{l}")
        qv = act.tile([P, MT_D, N], BF16, tag="qv", bufs=1, name=f"qv_{l}")
        qq = act.tile([P, MT_D, N], BF16, tag="qq", bufs=1, name=f"qq_{l}")

        def qkv_evac(base, dst, l=l):
            def f(mi, ps):
                m = base + mi
                nc.vector.tensor_scalar_add(
                    out=dst[:, mi, :], in0=ps,
                    scalar1=bat[:, l * MT_3D + m: l * MT_3D + m + 1])
            return f

        z1rhs = lambda ki, z1=z1: z1[:, ki, :]
        # k part (tiles 6..11)
        mm_phase(MT_D, KT_D,
                 lambda ki, mi, w=atk_t: w[ki][:, mi * P:(mi + 1) * P],
                 z1rhs, qkv_evac(6, qk), f"qk{l}")
        agk_in = dram.tile([KT_D, P, N], BF16, tag="agk_in", bufs=2,
                           name=f"agk_in_{l}")
        for c in range(2):
            nc.sync.dma_start(
                out=agk_in[3 * c:3 * c + 3].rearrange("kt p n -> p kt n"),
                in_=qk[:, 3 * c:3 * c + 3, :])
        agk_out = dram.tile([G, KT_D, P, N], BF16, tag="agk_out", bufs=2,
                            name=f"agk_out_{l}")
        nc.gpsimd.collective_compute(
            "AllGather", ALU.bypass, replica_groups=REPLICA_GROUPS,
            ins=[agk_in.opt()], outs=[agk_out.opt()])

        # v part (tiles 12..17), then transpose chunks + ones column
        mm_phase(MT_D, KT_D,
                 lambda ki, mi, w=atv_t: w[ki][:, mi * P:(mi + 1) * P],
                 z1rhs, qkv_evac(12, qv), f"qv{l}")
        vtc = act.tile([P, 2, H, DH + 1], BF16, tag="vtc", bufs=2, name=f"vtc_{l}")
        nc.vector.memset(vtc[:, :, :, DH:DH + 1], 1.0)
        for ft in range(KT_D):
            for tt in range(2):
                ps_t = psml.tile([P, P], BF16, tag="psml", bufs=2,
                                 name=f"ps_t_{l}_{ft}_{tt}")
                nc.tensor.transpose(ps_t, qv[:, ft, tt * P:(tt + 1) * P], ident)
                nc.vector.tensor_copy(out=vtc[:, tt, 2 * ft, 0:DH], in_=ps_t[:, 0:DH])
                nc.vector.tensor_copy(out=vtc[:, tt, 2 * ft + 1, 0:DH], in_=ps_t[:, DH:P])
        agv_in = dram.tile([2, P, H, DH + 1], BF16, tag="agv_in", bufs=2,
                           name=f"agv_in_{l}")
        for tt in range(2):
            nc.sync.dma_start(
                out=agv_in[tt].rearrange("p h c -> p (h c)"),
                in_=vtc[:, tt].rearrange("p h c -> p (h c)"))
        agv_out = dram.tile([G, 2, P, H, DH + 1], BF16, tag="agv_out", bufs=2,
                            name=f"agv_out_{l}")
        nc.gpsimd.collective_compute(
            "AllGather", ALU.bypass, replica_groups=REPLICA_GROUPS,
            ins=[agv_in.opt()], outs=[agv_out.opt()])

        # q part (tiles 0..5)
        mm_phase(MT_D, KT_D,
                 lambda ki, mi, w=atq_t: w[ki][:, mi * P:(mi + 1) * P],
                 z1rhs, qkv_evac(0, qq), f"qq{l}")

        # ---- gather K, V^T into SBUF (per-rank / per-keytile granularity)
        kf_r = []
        for r in range(G):
            t = act.tile([P, KT_D, N], BF16, tag="kf", bufs=4, name=f"kf_{l}_{r}")
            nc.sync.dma_start(out=t,
                              in_=agk_out[r].rearrange("kt p n -> p kt n"))
            kf_r.append(t)
        vtf_j = []
        for j in range(NKEY):
            t = act.tile([P, H, DH + 1], BF16, tag="vtf", bufs=8,
                         name=f"vtf_{l}_{j}")
            nc.sync.dma_start(out=t, in_=agv_out[j // 2, j % 2])
            vtf_j.append(t)

        # ---- attention: S^T/exp pipelined ahead of AV by LAG heads
        y = act.tile([P, KT_D, N], BF16, tag="y", bufs=2, name=f"y_{l}")

        def head_st(h, l=l, qq=qq, kf_r=kf_r):
            po = (h % 2) * DH
            ft = h // 2
            q_h = qq[po:po + DH, ft, :]
            expS = act.tile([P, NKEY, N], BF16, tag="expS", bufs=4,
                            name=f"expS_{l}_{h}")
            for j in range(NKEY):
                ps_s = pbig.tile([P, N], F32, tag="ps_mm", bufs=4,
                                 name=f"ps_s_{l}_{h}_{j}")
                k_h = kf_r[j // 2][po:po + DH, ft, (j % 2) * P:(j % 2) * P + P]
                nc.tensor.matmul(ps_s, k_h, q_h, start=True, stop=True)
                nc.scalar.activation(out=expS[:, j, :], in_=ps_s, func=AF.Exp)
            return expS

        def head_av(h, expS, l=l, y=y, vtf_j=vtf_j):
            po = (h % 2) * DH
            ft = h // 2
            ps_y = pbig.tile([DH + 1, N], F32, tag="ps_mm", bufs=4,
                             name=f"ps_y_{l}_{h}")
            for j in range(NKEY):
                nc.tensor.matmul(ps_y, vtf_j[j][:, h, :], expS[:, j, :],
                                 start=(j == 0), stop=(j == NKEY - 1))
            rec = st.tile([1, N], F32, tag="st", bufs=5, name=f"rec_{l}_{h}")
            nc.vector.reciprocal(out=rec, in_=ps_y[DH:DH + 1, :])
            rec_bf = st.tile([1, N], BF16, tag="stbf", bufs=2, name=f"recb_{l}_{h}")
            nc.vector.tensor_copy(out=rec_bf, in_=rec)
            ps_rb = pbig.tile([DH, N], F32, tag="ps_bc", bufs=2,
                              name=f"ps_rb_{l}_{h}")
            nc.tensor.matmul(ps_rb, ones_row_bf, rec_bf, start=True, stop=True)
            rb = act.tile([DH, N], BF16, tag="rb", bufs=2, name=f"rb_{l}_{h}")
            nc.scalar.activation(out=rb, in_=ps_rb, func=AF.Copy)
            nc.vector.tensor_tensor(out=y[po:po + DH, ft, :], in0=ps_y[0:DH, :],
                                    in1=rb, op=ALU.mult)

        LAG = 3
        pend = {}
        for h in range(H):
            pend[h] = head_st(h)
            if h >= LAG:
                head_av(h - LAG, pend.pop(h - LAG))
        for h in range(H - LAG, H):
            head_av(h, pend.pop(h))

        tap(f"y_{l}", y, BF16)

        # ---- attn out projection + residual: h2 = attnp(y) + b + h1
        h2 = act.tile([P, KT_D, N], F32, tag="h", bufs=2, name=f"h2_{l}")

        def ap_evac(mi, ps, l=l, h1=h1, h2=h2):
            nc.vector.scalar_tensor_tensor(
                out=h2[:, mi, :], in0=ps,
                scalar=bap[:, l * MT_D + mi: l * MT_D + mi + 1],
                in1=h1[:, mi, :], op0=ALU.add, op1=ALU.add)

        mm_phase(MT_D, KT_D,
                 lambda ki, mi, w=app_t: w[ki][:, mi * P:(mi + 1) * P],
                 lambda ki, y=y: y[:, ki, :],
                 ap_evac, f"ap{l}")

        tap(f"h2_{l}", h2, F32)

        # ---- ln2
        z2 = act.tile([P, KT_D, N], BF16, tag="z", bufs=2, name=f"z2_{l}")
        layer_norm(h2, KT_D, z2, f"ln2_{l}")

        # ---- MLP fc + gelu
        mg = act.tile([P, MT_MD, N], BF16, tag="mg", bufs=1, name=f"mg_{l}")

        def fc_evac(mi, ps, l=l, mg=mg):
            nc.scalar.activation(
                out=mg[:, mi, :], in_=ps, func=AF.Gelu_apprx_tanh,
                bias=bfc[:, l * MT_MD + mi: l * MT_MD + mi + 1])

        mm_phase(MT_MD, KT_D,
                 lambda ki, mi, fc_t=fc_t: fc_t[mi // MT_D][ki][:, (mi % MT_D) * P:(mi % MT_D + 1) * P],
                 lambda ki, z2=z2: z2[:, ki, :],
                 fc_evac, f"fc{l}")

        # ---- MLP proj + residual (h3 kept bf16; only feeds the down matmul)
        h3 = act.tile([P, KT_D, N], BF16, tag="h3", bufs=2, name=f"h3_{l}")

        def fp_evac(mi, ps, l=l, h2=h2, h3=h3):
            nc.vector.scalar_tensor_tensor(
                out=h3[:, mi, :], in0=ps,
                scalar=bfp[:, l * MT_D + mi: l * MT_D + mi + 1],
                in1=h2[:, mi, :], op0=ALU.add, op1=ALU.add)

        mm_phase(MT_D, KT_MD,
                 lambda ki, mi, w=fp_t: w[ki][:, mi * P:(mi + 1) * P],
                 lambda ki, mg=mg: mg[:, ki, :],
                 fp_evac, f"fp{l}")

        tap(f"mg_{l}", mg, BF16)
        tap(f"h3_{l}", h3, BF16)

        # ---- down projection + residual into x
        x_new = act.tile([P, KT_E, N], F32, tag="x", bufs=2, name=f"x_{l + 1}")

        def dn_evac(mi, ps, l=l, x=x, x_new=x_new):
            nc.vector.scalar_tensor_tensor(
                out=x_new[:, mi, :], in0=ps,
                scalar=bdn[:, l * MT_E + mi: l * MT_E + mi + 1],
                in1=x[:, mi, :], op0=ALU.add, op1=ALU.add)

        mm_phase(MT_E, KT_D,
                 lambda ki, mi, w=dn_t: w[ki][:, mi * P:(mi + 1) * P],
                 lambda ki, h3=h3: h3[:, ki, :],
                 dn_evac, f"dn{l}")
        tap(f"x_{l + 1}", x_new, F32)
        x = x_new

    # ---- final layernorm + small head
    zf = act.tile([P, KT_E, N], BF16, tag="z", bufs=2, name="zf")
    layer_norm(x, KT_E, zf, "lnf")
    sm_t = [load_w(io["w_sm"], 0, kt, 0, E, "wB", 47) for kt in range(KT_E)]
    osb = act.tile([P, MT_E, N], F32, tag="osb", bufs=1, name="osb")

    def sm_evac(mi, ps, osb=osb):
        nc.vector.tensor_scalar_add(out=osb[:, mi, :], in0=ps,
                                    scalar1=bsm[:, mi:mi + 1])

    mm_phase(MT_E, KT_E,
             lambda ki, mi, w=sm_t: w[ki][:, mi * P:(mi + 1) * P],
             lambda ki, zf=zf: zf[:, ki, :],
             sm_evac, "sm")
    nc.sync.dma_start(out=io["outT"].rearrange("(m p) n -> p m n", p=P), in_=osb)

    for pool in (psml, pbig, dram, st, act, wgt, con):
        pool.release()


# ---------------------------------------------------------------------------
# host side
# ---------------------------------------------------------------------------

def _erf(x):
    return np.vectorize(math.erf)(x.astype(np.float64)).astype(np.float32)


def _prepare(inputs):
    """Host preprocessing: fold embeddings + LN gains, cast weights to bf16."""
    ie = _f32(inputs["input_embeddings"])
    wpe = _f32(inputs["wpe"])
    ts = _f32(inputs["time_step"])
    time_w = _f32(inputs["time_w"])
    time_b = _f32(inputs["time_b"])

    half = E // 2
    freqs = np.exp(np.arange(half, dtype=np.float32) * (-math.log(10000.0) / (half - 1)))
    emb = ts[:, None] * freqs[None, :]
    emb = np.concatenate([np.sin(emb), np.cos(emb)], axis=-1).astype(np.float32)
    pre = emb @ time_w + time_b
    te = pre * 0.5 * (1.0 + _erf(pre / np.sqrt(2.0)))
    x0 = ie + wpe[None] + te[:, None, :]           # [B, T, E]

    ln1_g, ln1_b = _f32(inputs["ln1_g"]), _f32(inputs["ln1_b"])
    ln2_g, ln2_b = _f32(inputs["ln2_g"]), _f32(inputs["ln2_b"])
    lnf_g, lnf_b = _f32(inputs["lnf_g"]), _f32(inputs["lnf_b"])
    attn_w, attn_b = _f32(inputs["attn_w"]), _f32(inputs["attn_b"])
    fc_w, fc_b = _f32(inputs["fc_w"]), _f32(inputs["fc_b"])
    small_w, small_b = _f32(inputs["small_w"]), _f32(inputs["small_b"])

    # fold LN gain/bias into the following matmul; scale q by 1/sqrt(DH)
    attn_wf = ln1_g[:, :, None] * attn_w
    attn_bf = attn_b + np.einsum("ld,ldo->lo", ln1_b, attn_w)
    attn_wf[:, :, :D] *= 1.0 / math.sqrt(DH)
    attn_bf[:, :D] *= 1.0 / math.sqrt(DH)
    fc_wf = ln2_g[:, :, None] * fc_w
    fc_bf = fc_b + np.einsum("ld,ldo->lo", ln2_b, fc_w)
    small_wf = lnf_g[:, None] * small_w
    small_bf = small_b + lnf_b @ small_w

    feeds = {
        "w_up": _bf(inputs["up_w"]), "b_up": _f32(inputs["up_b"]),
        "w_at": _bf(attn_wf), "b_at": _f32(attn_bf),
        "w_ap": _bf(inputs["attnp_w"]), "b_ap": _f32(inputs["attnp_b"]),
        "w_fc": _bf(fc_wf), "b_fc": _f32(fc_bf),
        "w_fp": _bf(inputs["fcp_w"]), "b_fp": _f32(inputs["fcp_b"]),
        "w_dn": _bf(inputs["down_w"]), "b_dn": _f32(inputs["down_b"]),
        "w_sm": _bf(small_wf), "b_sm": _f32(small_bf),
    }
    return x0, feeds


def _get_program():
    if "nc" not in _CACHE:
        _CACHE["nc"] = build_program()
    return _CACHE["nc"]


def make_in_maps(inputs):
    x0, feeds = _prepare(inputs)
    in_maps = []
    for c in range(NC):
        b, s = c // G, (c % G) * N
        xT = np.ascontiguousarray(x0[b, s:s + N, :].T)     # [E, N] f32
        in_maps.append({"xT": xT, **feeds})
    return in_maps


def assemble(results):
    out = np.empty((B, T, E), dtype=np.float32)
    for c in range(NC):
        b, s = c // G, (c % G) * N
        out[b, s:s + N, :] = results[c]["outT"].T
    return out


def kernel(**inputs) -> np.ndarray:
    nc = _get_program()
    in_maps = make_in_maps(inputs)
    res = bass_utils.run_bass_kernel_spmd(nc, in_maps, core_ids=list(range(NC)))
    return assemble(res.results)


# revision 15
# speedup vs baseline: 1.0483x; 1.0291x over previous
"""Trainium2 Bass kernel for nn_Denoiser (6-layer dense transformer denoiser).

Strategy (8 NeuronCores):
  - Token-parallel: the 2048 tokens (B=2 x T=1024) are split 256/core.
    Cores 0-3 hold batch 0, cores 4-7 hold batch 1.
  - All per-token compute (matmuls, layernorms, MLP) is local to a core.
    Attention needs all keys/values of the core's batch, so each layer does
    one grouped AllGather of K plus one of V^T among the 4 cores of a batch.
  - Weights are replicated per core, pre-cast to bf16 on the host and
    streamed from HBM.  Activations stay SBUF-resident in [feature, token]
    layout so matmuls chain without transposes (out = W.T @ x_T).
  - LayerNorm gains/biases are folded into the following weight on the host;
    on-device LN is (x - mu) * rsig with stats from ones-vector matmuls
    (partition-dim reductions on the PE).
  - Softmax skips max-subtraction (scores are in [-3, 3] for this model) and
    gets its denominator for free from a ones-column appended to V^T.
"""
import sys

sys.path.insert(0, "/opt/trn_rl_repo")

import math

import numpy as np
import ml_dtypes

import concourse.bass as bass
import concourse.bacc as bacc
import concourse.tile as tile
from concourse import mybir
from concourse import bass_utils
from concourse.masks import make_identity

F32 = mybir.dt.float32
BF16 = mybir.dt.bfloat16
AF = mybir.ActivationFunctionType
ALU = mybir.AluOpType

B, T, L = 2, 1024, 6
E, D, H, M = 512, 768, 12, 4
DH = D // H          # 64
N = 256              # tokens per core
P = 128
NC = 8               # cores
G = 4                # cores per batch group
KT_E, KT_D, KT_MD = E // P, D // P, (M * D) // P   # 4, 6, 24
MT_D, MT_3D, MT_MD, MT_E = D // P, 3 * D // P, (M * D) // P, E // P  # 6, 18, 24, 4
NKEY = T // P        # 8 key tiles of 128
REPLICA_GROUPS = [[0, 1, 2, 3], [4, 5, 6, 7]]

_CACHE = {}
DEBUG_TAPS = False


def _bf(x):
    return np.ascontiguousarray(x).astype(ml_dtypes.bfloat16)


def _f32(x):
    return np.ascontiguousarray(x).astype(np.float32)


# ---------------------------------------------------------------------------
# device program
# ---------------------------------------------------------------------------

def build_program():
    nc = bacc.Bacc(
        "TRN2", target_bir_lowering=False, debug=False,
        enable_asserts=True, num_devices=NC,
    )

    xT = nc.dram_tensor("xT", [E, N], F32, kind="ExternalInput").ap()
    w_up = nc.dram_tensor("w_up", [L, E, D], BF16, kind="ExternalInput").ap()
    b_up = nc.dram_tensor("b_up", [L, D], F32, kind="ExternalInput").ap()
    w_at = nc.dram_tensor("w_at", [L, D, 3 * D], BF16, kind="ExternalInput").ap()
    b_at = nc.dram_tensor("b_at", [L, 3 * D], F32, kind="ExternalInput").ap()
    w_ap = nc.dram_tensor("w_ap", [L, D, D], BF16, kind="ExternalInput").ap()
    b_ap = nc.dram_tensor("b_ap", [L, D], F32, kind="ExternalInput").ap()
    w_fc = nc.dram_tensor("w_fc", [L, D, M * D], BF16, kind="ExternalInput").ap()
    b_fc = nc.dram_tensor("b_fc", [L, M * D], F32, kind="ExternalInput").ap()
    w_fp = nc.dram_tensor("w_fp", [L, M * D, D], BF16, kind="ExternalInput").ap()
    b_fp = nc.dram_tensor("b_fp", [L, D], F32, kind="ExternalInput").ap()
    w_dn = nc.dram_tensor("w_dn", [L, D, E], BF16, kind="ExternalInput").ap()
    b_dn = nc.dram_tensor("b_dn", [L, E], F32, kind="ExternalInput").ap()
    w_sm = nc.dram_tensor("w_sm", [E, E], BF16, kind="ExternalInput").ap()
    b_sm = nc.dram_tensor("b_sm", [E], F32, kind="ExternalInput").ap()
    outT = nc.dram_tensor("outT", [E, N], F32, kind="ExternalOutput").ap()

    with tile.TileContext(nc) as tc:
        _body(tc, dict(
            xT=xT, w_up=w_up, b_up=b_up, w_at=w_at, b_at=b_at, w_ap=w_ap,
            b_ap=b_ap, w_fc=w_fc, b_fc=b_fc, w_fp=w_fp, b_fp=b_fp,
            w_dn=w_dn, b_dn=b_dn, w_sm=w_sm, b_sm=b_sm, outT=outT,
        ))
    nc.compile()
    return nc


def _body(tc, io):
    nc = tc.nc

    con = tc.alloc_tile_pool(name="con", bufs=1)           # constants / biases
    wgt = tc.alloc_tile_pool(name="wgt", bufs=1)           # streamed weights
    act = tc.alloc_tile_pool(name="act", bufs=1)           # activations
    st = tc.alloc_tile_pool(name="st", bufs=1)             # [1,256] stats
    dram = tc.alloc_tile_pool(name="dram", bufs=1, space="DRAM")
    pbig = tc.alloc_tile_pool(name="pbig", bufs=1, space="PSUM")
    psml = tc.alloc_tile_pool(name="psml", bufs=1, space="PSUM")

    # --- constants ----------------------------------------------------------
    ident = con.tile([P, P], BF16, tag="ident", name="ident")
    make_identity(nc, ident)
    ones_col = con.tile([P, 1], BF16, tag="ones_col", name="ones_col")
    nc.vector.memset(ones_col, 1.0)
    ones_row = con.tile([1, P], F32, tag="ones_row", name="ones_row")
    nc.vector.memset(ones_row, 1.0)
    ones_row_bf = con.tile([1, DH], BF16, tag="ones_row_bf", name="ones_row_bf")
    nc.vector.memset(ones_row_bf, 1.0)
    eps = con.tile([1, 1], F32, tag="eps", name="eps")
    nc.vector.memset(eps, 1e-5)

    # --- biases (resident, f32) --------------------------------------------
    def load_bias(name, dram_ap, nm):
        t = con.tile([P, L * nm], F32, tag=name, name=name)
        nc.sync.dma_start(out=t, in_=dram_ap.rearrange("l (m p) -> p (l m)", p=P))
        return t

    bup = load_bias("bup", io["b_up"], MT_D)
    bat = load_bias("bat", io["b_at"], MT_3D)
    bap = load_bias("bap", io["b_ap"], MT_D)
    bfc = load_bias("bfc", io["b_fc"], MT_MD)
    bfp = load_bias("bfp", io["b_fp"], MT_D)
    bdn = load_bias("bdn", io["b_dn"], MT_E)
    bsm = con.tile([P, MT_E], F32, tag="bsm", name="bsm")
    nc.sync.dma_start(out=bsm, in_=io["b_sm"].rearrange("(m p) -> p m", p=P))

    # --- weight streaming ---------------------------------------------------
    def load_w(dram3, l, kt, c0, c1, tag, bufs):
        t = wgt.tile([P, c1 - c0], BF16, tag=tag, bufs=bufs,
                     name=f"{tag}_{l}_{kt}_{c0}")
        d2 = dram3[l] if len(dram3.shape) == 3 else dram3
        src = d2.rearrange("(kt p) n -> kt p n", p=P)[kt, :, c0:c1]
        nc.sync.dma_start(out=t, in_=src)
        return t

    # --- LN helper ----------------------------------------------------------
    def layer_norm(src, nt, z, tag):
        """z (bf16 [P, nt, N]) = (src - mu) * rsig, stats over nt*P features."""
        ps_sum = psml.tile([1, N], F32, tag="psml", bufs=2, name=f"ps_sum_{tag}")
        ps_sq = psml.tile([1, N], F32, tag="psml", bufs=2, name=f"ps_sq_{tag}")
        for kt in range(nt):
            hb = act.tile([P, N], BF16, tag="lnb", bufs=2, name=f"lnb_{tag}_{kt}")
            nc.vector.tensor_copy(out=hb, in_=src[:, kt, :])
            nc.tensor.matmul(ps_sum, ones_col, hb, start=(kt == 0), stop=(kt == nt - 1))
            sq = act.tile([P, N], BF16, tag="lnsq", bufs=2, name=f"lnsq_{tag}_{kt}")
            nc.scalar.activation(out=sq, in_=hb, func=AF.Square)
            nc.tensor.matmul(ps_sq, ones_col, sq, start=(kt == 0), stop=(kt == nt - 1))
        dinv = 1.0 / (nt * P)
        mu = st.tile([1, N], F32, tag="st", bufs=5, name=f"mu_{tag}")
        nc.vector.tensor_scalar_mul(out=mu, in0=ps_sum, scalar1=dinv)
        ms = st.tile([1, N], F32, tag="st", bufs=5, name=f"ms_{tag}")
        nc.vector.tensor_scalar_mul(out=ms, in0=ps_sq, scalar1=dinv)
        # var = ms - mu^2
        var = st.tile([1, N], F32, tag="st", bufs=5, name=f"var_{tag}")
        mu2 = st.tile([1, N], F32, tag="st", bufs=5, name=f"mu2_{tag}")
        nc.vector.tensor_mul(out=mu2, in0=mu, in1=mu)
        nc.vector.tensor_sub(out=var, in0=ms, in1=mu2)
        sd = st.tile([1, N], F32, tag="st", bufs=5, name=f"sd_{tag}")
        nc.scalar.activation(out=sd, in_=var, func=AF.Sqrt, bias=eps)
        rsig = st.tile([1, N], F32, tag="st", bufs=5, name=f"rsig_{tag}")
        nc.vector.reciprocal(out=rsig, in_=sd)
        ps_rs = pbig.tile([P, N], F32, tag="ps_bc", bufs=2, name=f"ps_rs_{tag}")
        nc.tensor.matmul(ps_rs, ones_row, rsig, start=True, stop=True)
        ps_nm = pbig.tile([P, N], F32, tag="ps_bc", bufs=2, name=f"ps_nm_{tag}")
        nc.tensor.matmul(ps_nm, ones_row, mu, start=True, stop=True)
        for kt in range(nt):
            tmp = act.tile([P, N], F32, tag="lntmp", bufs=2, name=f"lnt_{tag}_{kt}")
            nc.vector.tensor_sub(out=tmp, in0=src[:, kt, :], in1=ps_nm)
            nc.vector.tensor_mul(out=z[:, kt, :], in0=tmp, in1=ps_rs)

    # --- generic matmul phase ----------------------------------------------
    def mm_phase(n_m, n_k, lhsT_fn, rhs_fn, evac_fn, psname):
        for mi in range(n_m):
            ps = pbig.tile([P, N], F32, tag="ps_mm", bufs=4,
                           name=f"ps_{psname}_{mi}")
            for ki in range(n_k):
                nc.tensor.matmul(ps, lhsT_fn(ki, mi), rhs_fn(ki),
                                 start=(ki == 0), stop=(ki == n_k - 1))
            evac_fn(mi, ps)


    def tap(name, t, dtype):
        if not DEBUG_TAPS:
            return
        rows = t.shape[1] * P
        d = nc.dram_tensor(f"dbg_{name}", [rows, N], dtype,
                           kind="ExternalOutput").ap()
        nc.sync.dma_start(out=d.rearrange("(kt p) n -> p kt n", p=P), in_=t)

    # --- input --------------------------------------------------------------
    x = act.tile([P, KT_E, N], F32, tag="x", bufs=2, name="x0")
    nc.sync.dma_start(out=x, in_=io["xT"].rearrange("(kt p) n -> p kt n", p=P))

    for l in range(L):
        # ---- weights for up + qkv-k (rest loaded phase-locally below)
        up_t = [load_w(io["w_up"], l, kt, 0, D, "wB", 47) for kt in range(KT_E)]
        atk_t = [load_w(io["w_at"], l, kt, D, 2 * D, "wB", 47) for kt in range(KT_D)]

        # ---- x cast to bf16
        xb = act.tile([P, KT_E, N], BF16, tag="xb", bufs=2, name=f"xb_{l}")
        nc.vector.tensor_copy(out=xb, in_=x)

        # ---- up projection: h1 = up(x) + b
        h1 = act.tile([P, KT_D, N], F32, tag="h", bufs=2, name=f"h1_{l}")

        def up_evac(mi, ps, l=l, h1=h1):
            nc.vector.tensor_scalar_add(
                out=h1[:, mi, :], in0=ps, scalar1=bup[:, l * MT_D + mi: l * MT_D + mi + 1])

        mm_phase(MT_D, KT_E,
                 lambda ki, mi, up_t=up_t: up_t[ki][:, mi * P:(mi + 1) * P],
                 lambda ki, xb=xb: xb[:, ki, :],
                 up_evac, f"up{l}")

        tap(f"h1_{l}", h1, F32)

        # ---- ln1
        z1 = act.tile([P, KT_D, N], BF16, tag="z", bufs=2, name=f"z1_{l}")
        layer_norm(h1, KT_D, z1, f"ln1_{l}")
        tap(f"z1_{l}", z1, BF16)

        # ---- qkv (k first, then v, then q) + AllGathers
        qk = act.tile([P, MT_D, N], BF16, tag="qk", bufs=1, name=f"qk_{l}")
        qv = act.tile([P, MT_D, N], BF16, tag="qv", bufs=1, name=f"qv_{l}")
        qq = act.tile([P, MT_D, N], BF16, tag="qq", bufs=1, name=f"qq_{l}")

        def qkv_evac(base, dst, l=l):
            def f(mi, ps):
                m = base + mi
                nc.vector.tensor_scalar_add(
                    out=dst[:, mi, :], in0=ps,
                    scalar1=bat[:, l * MT_3D + m: l * MT_3D + m + 1])
            return f

        z1rhs = lambda ki, z1=z1: z1[:, ki, :]
        # k part (tiles 6..11)
        mm_phase(MT_D, KT_D,
                 lambda ki, mi, w=atk_t: w[ki][:, mi * P:(mi + 1) * P],
                 z1rhs, qkv_evac(6, qk), f"qk{l}")
        agk_in = dram.tile([KT_D, P, N], BF16, tag="agk_in", bufs=2,
                           name=f"agk_in_{l}")
        for c in range(2):
            nc.sync.dma_start(
                out=agk_in[3 * c:3 * c + 3].rearrange("kt p n -> p kt n"),
                in_=qk[:, 3 * c:3 * c + 3, :])
        agk_out = dram.tile([G, KT_D, P, N], BF16, tag="agk_out", bufs=2,
                            name=f"agk_out_{l}")
        nc.gpsimd.collective_compute(
            "AllGather", ALU.bypass, replica_groups=REPLICA_GROUPS,
            ins=[agk_in.opt()], outs=[agk_out.opt()])

        # v part (tiles 12..17), then transpose chunks + ones column
        atv_t = [load_w(io["w_at"], l, kt, 2 * D, 3 * D, "wB", 47) for kt in range(KT_D)]
        mm_phase(MT_D, KT_D,
                 lambda ki, mi, w=atv_t: w[ki][:, mi * P:(mi + 1) * P],
                 z1rhs, qkv_evac(12, qv), f"qv{l}")
        vtc = act.tile([P, 2, H, DH + 1], BF16, tag="vtc", bufs=2, name=f"vtc_{l}")
        nc.vector.memset(vtc[:, :, :, DH:DH + 1], 1.0)
        for ft in range(KT_D):
            for tt in range(2):
                ps_t = psml.tile([P, P], BF16, tag="psml", bufs=2,
                                 name=f"ps_t_{l}_{ft}_{tt}")
                nc.tensor.transpose(ps_t, qv[:, ft, tt * P:(tt + 1) * P], ident)
                nc.vector.tensor_copy(out=vtc[:, tt, 2 * ft, 0:DH], in_=ps_t[:, 0:DH])
                nc.vector.tensor_copy(out=vtc[:, tt, 2 * ft + 1, 0:DH], in_=ps_t[:, DH:P])
        agv_in = dram.tile([2, P, H, DH + 1], BF16, tag="agv_in", bufs=2,
                           name=f"agv_in_{l}")
        for tt in range(2):
            nc.sync.dma_start(
                out=agv_in[tt].rearrange("p h c -> p (h c)"),
                in_=vtc[:, tt].rearrange("p h c -> p (h c)"))
        agv_out = dram.tile([G, 2, P, H, DH + 1], BF16, tag="agv_out", bufs=2,
                            name=f"agv_out_{l}")
        nc.gpsimd.collective_compute(
            "AllGather", ALU.bypass, replica_groups=REPLICA_GROUPS,
            ins=[agv_in.opt()], outs=[agv_out.opt()])

        # q part (tiles 0..5)
        atq_t = [load_w(io["w_at"], l, kt, 0, D, "wB", 47) for kt in range(KT_D)]
        mm_phase(MT_D, KT_D,
                 lambda ki, mi, w=atq_t: w[ki][:, mi * P:(mi + 1) * P],
                 z1rhs, qkv_evac(0, qq), f"qq{l}")

        # ---- gather K, V^T into SBUF (per-rank / per-keytile granularity)
        kf_r = []
        for r in range(G):
            t = act.tile([P, KT_D, N], BF16, tag="kf", bufs=4, name=f"kf_{l}_{r}")
            nc.scalar.dma_start(out=t,
                                in_=agk_out[r].rearrange("kt p n -> p kt n"))
            kf_r.append(t)
        vtf_j = []
        for j in range(NKEY):
            t = act.tile([P, H, DH + 1], BF16, tag="vtf", bufs=8,
                         name=f"vtf_{l}_{j}")
            nc.scalar.dma_start(out=t, in_=agv_out[j // 2, j % 2])
            vtf_j.append(t)

        # ---- attention: S^T/exp pipelined ahead of AV by LAG heads
        y = act.tile([P, KT_D, N], BF16, tag="y", bufs=2, name=f"y_{l}")

        def head_st(h, l=l, qq=qq, kf_r=kf_r):
            po = (h % 2) * DH
            ft = h // 2
            q_h = qq[po:po + DH, ft, :]
            expS = act.tile([P, NKEY, N], BF16, tag="expS", bufs=4,
                            name=f"expS_{l}_{h}")
            for j in range(NKEY):
                ps_s = pbig.tile([P, N], F32, tag="ps_mm", bufs=4,
                                 name=f"ps_s_{l}_{h}_{j}")
                k_h = kf_r[j // 2][po:po + DH, ft, (j % 2) * P:(j % 2) * P + P]
                nc.tensor.matmul(ps_s, k_h, q_h, start=True, stop=True)
                nc.scalar.activation(out=expS[:, j, :], in_=ps_s, func=AF.Exp)
            return expS

        def head_av(h, expS, l=l, y=y, vtf_j=vtf_j):
            po = (h % 2) * DH
            ft = h // 2
            ps_y = pbig.tile([DH + 1, N], F32, tag="ps_mm", bufs=4,
                             name=f"ps_y_{l}_{h}")
            for j in range(NKEY):
                nc.tensor.matmul(ps_y, vtf_j[j][:, h, :], expS[:, j, :],
                                 start=(j == 0), stop=(j == NKEY - 1))
            rec = st.tile([1, N], F32, tag="st", bufs=5, name=f"rec_{l}_{h}")
            nc.vector.reciprocal(out=rec, in_=ps_y[DH:DH + 1, :])
            rec_bf = st.tile([1, N], BF16, tag="stbf", bufs=2, name=f"recb_{l}_{h}")
            nc.vector.tensor_copy(out=rec_bf, in_=rec)
            ps_rb = pbig.tile([DH, N], F32, tag="ps_bc", bufs=2,
                              name=f"ps_rb_{l}_{h}")
            nc.tensor.matmul(ps_rb, ones_row_bf, rec_bf, start=True, stop=True)
            rb = act.tile([DH, N], BF16, tag="rb", bufs=2, name=f"rb_{l}_{h}")
            nc.scalar.activation(out=rb, in_=ps_rb, func=AF.Copy)
            nc.vector.tensor_tensor(out=y[po:po + DH, ft, :], in0=ps_y[0:DH, :],
                                    in1=rb, op=ALU.mult)

        LAG = 3
        pend = {}
        for h in range(H):
            pend[h] = head_st(h)
            if h >= LAG:
                head_av(h - LAG, pend.pop(h - LAG))
        for h in range(H - LAG, H):
            head_av(h, pend.pop(h))

        tap(f"y_{l}", y, BF16)

        # ---- attn out projection + residual: h2 = attnp(y) + b + h1
        app_t = [load_w(io["w_ap"], l, kt, 0, D, "wB", 47) for kt in range(KT_D)]
        fc_t = [[load_w(io["w_fc"], l, kt, c * D, (c + 1) * D, "wB", 47)
                 for kt in range(KT_D)] for c in range(4)]
        h2 = act.tile([P, KT_D, N], F32, tag="h", bufs=2, name=f"h2_{l}")

        def ap_evac(mi, ps, l=l, h1=h1, h2=h2):
            nc.vector.scalar_tensor_tensor(
                out=h2[:, mi, :], in0=ps,
                scalar=bap[:, l * MT_D + mi: l * MT_D + mi + 1],
                in1=h1[:, mi, :], op0=ALU.add, op1=ALU.add)

        mm_phase(MT_D, KT_D,
                 lambda ki, mi, w=app_t: w[ki][:, mi * P:(mi + 1) * P],
                 lambda ki, y=y: y[:, ki, :],
                 ap_evac, f"ap{l}")

        tap(f"h2_{l}", h2, F32)

        # ---- ln2
        z2 = act.tile([P, KT_D, N], BF16, tag="z", bufs=2, name=f"z2_{l}")
        layer_norm(h2, KT_D, z2, f"ln2_{l}")

        # ---- MLP fc + gelu
        mg = act.tile([P, MT_MD, N], BF16, tag="mg", bufs=1, name=f"mg_{l}")

        def fc_evac(mi, ps, l=l, mg=mg):
            nc.scalar.activation(
                out=mg[:, mi, :], in_=ps, func=AF.Gelu_apprx_tanh,
                bias=bfc[:, l * MT_MD + mi: l * MT_MD + mi + 1])

        mm_phase(MT_MD, KT_D,
                 lambda ki, mi, fc_t=fc_t: fc_t[mi // MT_D][ki][:, (mi % MT_D) * P:(mi % MT_D + 1) * P],
                 lambda ki, z2=z2: z2[:, ki, :],
                 fc_evac, f"fc{l}")

        # ---- MLP proj + residual (h3 kept bf16; only feeds the down matmul)
        fp_t = [load_w(io["w_fp"], l, kt, 0, D, "wB", 47) for kt in range(KT_MD)]
        h3 = act.tile([P, KT_D, N], BF16, tag="h3", bufs=2, name=f"h3_{l}")

        def fp_evac(mi, ps, l=l, h2=h2, h3=h3):
            nc.vector.scalar_tensor_tensor(
                out=h3[:, mi, :], in0=ps,
                scalar=bfp[:, l * MT_D + mi: l * MT_D + mi + 1],
                in1=h2[:, mi, :], op0=ALU.add, op1=ALU.add)

        mm_phase(MT_D, KT_MD,
                 lambda ki, mi, w=fp_t: w[ki][:, mi * P:(mi + 1) * P],
                 lambda ki, mg=mg: mg[:, ki, :],
                 fp_evac, f"fp{l}")

        tap(f"mg_{l}", mg, BF16)
        tap(f"h3_{l}", h3, BF16)

        # ---- down projection + residual into x
        dn_t = [load_w(io["w_dn"], l, kt, 0, E, "wB", 47) for kt in range(KT_D)]
        x_new = act.tile([P, KT_E, N], F32, tag="x", bufs=2, name=f"x_{l + 1}")

        def dn_evac(mi, ps, l=l, x=x, x_new=x_new):
            nc.vector.scalar_tensor_tensor(
                out=x_new[:, mi, :], in0=ps,
                scalar=bdn[:, l * MT_E + mi: l * MT_E + mi + 1],
                in1=x[:, mi, :], op0=ALU.add, op1=ALU.add)

        mm_phase(MT_E, KT_D,
                 lambda ki, mi, w=dn_t: w[ki][:, mi * P:(mi + 1) * P],
                 lambda ki, h3=h3: h3[:, ki, :],
                 dn_evac, f"dn{l}")
        tap(f"x_{l + 1}", x_new, F32)
        x = x_new

    # ---- final layernorm + small head
    zf = act.tile([P, KT_E, N], BF16, tag="z", bufs=2, name="zf")
    layer_norm(x, KT_E, zf, "lnf")
    sm_t = [load_w(io["w_sm"], 0, kt, 0, E, "wB", 47) for kt in range(KT_E)]
    osb = act.tile([P, MT_E, N], F32, tag="osb", bufs=1, name="osb")

    def sm_evac(mi, ps, osb=osb):
        nc.vector.tensor_scalar_add(out=osb[:, mi, :], in0=ps,
                                    scalar1=bsm[:, mi:mi + 1])

    mm_phase(MT_E, KT_E,
             lambda ki, mi, w=sm_t: w[ki][:, mi * P:(mi + 1) * P],
             lambda ki, zf=zf: zf[:, ki, :],
             sm_evac, "sm")
    nc.sync.dma_start(out=io["outT"].rearrange("(m p) n -> p m n", p=P), in_=osb)

    for pool in (psml, pbig, dram, st, act, wgt, con):
        pool.release()


# ---------------------------------------------------------------------------
# host side
# ---------------------------------------------------------------------------

def _erf(x):
    return np.vectorize(math.erf)(x.astype(np.float64)).astype(np.float32)


def _prepare(inputs):
    """Host preprocessing: fold embeddings + LN gains, cast weights to bf16."""
    ie = _f32(inputs["input_embeddings"])
    wpe = _f32(inputs["wpe"])
    ts = _f32(inputs["time_step"])
    time_w = _f32(inputs["time_w"])
    time_b = _f32(inputs["time_b"])

    half = E // 2
    freqs = np.exp(np.arange(half, dtype=np.float32) * (-math.log(10000.0) / (half - 1)))
    emb = ts[:, None] * freqs[None, :]
    emb = np.concatenate([np.sin(emb), np.cos(emb)], axis=-1).astype(np.float32)
    pre = emb @ time_w + time_b
    te = pre * 0.5 * (1.0 + _erf(pre / np.sqrt(2.0)))
    x0 = ie + wpe[None] + te[:, None, :]           # [B, T, E]

    ln1_g, ln1_b = _f32(inputs["ln1_g"]), _f32(inputs["ln1_b"])
    ln2_g, ln2_b = _f32(inputs["ln2_g"]), _f32(inputs["ln2_b"])
    lnf_g, lnf_b = _f32(inputs["lnf_g"]), _f32(inputs["lnf_b"])
    attn_w, attn_b = _f32(inputs["attn_w"]), _f32(inputs["attn_b"])
    fc_w, fc_b = _f32(inputs["fc_w"]), _f32(inputs["fc_b"])
    small_w, small_b = _f32(inputs["small_w"]), _f32(inputs["small_b"])

    # fold LN gain/bias into the following matmul; scale q by 1/sqrt(DH)
    attn_wf = ln1_g[:, :, None] * attn_w
    attn_bf = attn_b + np.einsum("ld,ldo->lo", ln1_b, attn_w)
    attn_wf[:, :, :D] *= 1.0 / math.sqrt(DH)
    attn_bf[:, :D] *= 1.0 / math.sqrt(DH)
    fc_wf = ln2_g[:, :, None] * fc_w
    fc_bf = fc_b + np.einsum("ld,ldo->lo", ln2_b, fc_w)
    small_wf = lnf_g[:, None] * small_w
    small_bf = small_b + lnf_b @ small_w

    feeds = {
        "w_up": _bf(inputs["up_w"]), "b_up": _f32(inputs["up_b"]),
        "w_at": _bf(attn_wf), "b_at": _f32(attn_bf),
        "w_ap": _bf(inputs["attnp_w"]), "b_ap": _f32(inputs["attnp_b"]),
        "w_fc": _bf(fc_wf), "b_fc": _f32(fc_bf),
        "w_fp": _bf(inputs["fcp_w"]), "b_fp": _f32(inputs["fcp_b"]),
        "w_dn": _bf(inputs["down_w"]), "b_dn": _f32(inputs["down_b"]),
        "w_sm": _bf(small_wf), "b_sm": _f32(small_bf),
    }
    return x0, feeds


def _get_program():
    if "nc" not in _CACHE:
        _CACHE["nc"] = build_program()
    return _CACHE["nc"]


def make_in_maps(inputs):
    x0, feeds = _prepare(inputs)
    in_maps = []
    for c in range(NC):
        b, s = c // G, (c % G) * N
        xT = np.ascontiguousarray(x0[b, s:s + N, :].T)     # [E, N] f32
        in_maps.append({"xT": xT, **feeds})
    return in_maps


def assemble(results):
    out = np.empty((B, T, E), dtype=np.float32)
    for c in range(NC):
        b, s = c // G, (c % G) * N
        out[b, s:s + N, :] = results[c]["outT"].T
    return out


def kernel(**inputs) -> np.ndarray:
    nc = _get_program()
    in_maps = make_in_maps(inputs)
    res = bass_utils.run_bass_kernel_spmd(nc, in_maps, core_ids=list(range(NC)))
    return assemble(res.results)
